# revision 1
# baseline (speedup 1.0000x reference)
"""Trainium2 Bass kernel for nn_Aliformer (dense transformer w/ knowledge attention).

Math (reference, B=4 L=1024 DM=512 DF=1024 H=8 DK=128):
  v/k/q       = x @ {Wv,Wk,Wq}.T + b            (B,L,1024)
  k_fei/q_fei = x_knowledge @ {Wkk,Wkq}.T + b   (B,L,1024)
  q,k,qf,kf   = second linear (1024->1024), then torch-style reshape
                (B,L,1024)->(B,8,1024,128) WITHOUT transpose.
  att  = (q@k^T + qf@kf^T)/sqrt(256); score = softmax(att)
  out  = (score @ v-reshaped) -> (B,L,1024); final = out @ Wout.T + bout

Key structural fact: the no-transpose reshape means head h of batch b only
touches rows [h*128,(h+1)*128) of the flattened (4096, ·) activations, so the
whole network decomposes into 32 independent 128-row blocks. Each of the 8
cores processes 4 contiguous blocks (512 rows) with zero communication.

Within the attention of a block, row index l' = 8r + c (r = row-in-block,
c = feature chunk). We compute everything in the permuted order i' = c*128+r
(softmax is permutation-invariant over the full axis), which makes every
operand a natural SBUF tile slice:
  Q'.T chunk = q2T (feature-major) slice, V' chunk = v (row-major) slice.

Layouts on device (per core, R=512 rows):
  activations feature-major ("T"): tiles (128 part = feat-in-chunk, rows free)
  v row-major: (128 part = row-in-block, feat free)
  attT[(cj,rj), i'] per block: psum (128, 1024); softmax along partitions+chunks
  done via incremental DVE adds + 7-step partition tree + reciprocal +
  gpsimd partition_broadcast; division folded into the PSUM->SBUF move of the
  attention output.

Projection/fc matmuls run in float32r (full-rate PE mode; fp32 with 11-bit
mantissa). Walrus requires matmul operands to be *produced* as fp32r, so every
matmul-feeding DMA/ACT/DVE op writes through an fp32r-bitcast AP (the bits
remain valid fp32; non-matmul readers read them as plain f32). Weights are
pre-transposed AND pre-rounded to fp32r on the host. Attention-side tensors
(q2/k2/qf2/kf2, v, expT) are stored bf16 to fit SBUF (emulated end-to-end
error 2.6e-3 vs reference). bout is added on the host (it commutes past the
final matmul); bv cannot be (softmax partial sums weight it) and is added on
device via a DMA-broadcast bias tile.
"""

import sys

for _p in ("/opt/trn_rl_repo", "/root/.axon_site/_ro/trn_rl_repo"):
    if _p not in sys.path:
        sys.path.insert(0, _p)

import numpy as np

import concourse.bass as bass
import concourse.mybir as mybir
import concourse.tile as tile
from concourse import bacc
from concourse.bass_utils import run_bass_kernel_spmd

F32 = mybir.dt.float32
F32R = mybir.dt.float32r
BF16 = mybir.dt.bfloat16
EXP = mybir.ActivationFunctionType.Exp
IDENT = mybir.ActivationFunctionType.Identity

N_CORES = 8
R = 512          # rows per core
NB = 4           # 128-row blocks per core
DIN = 512        # model dim (input of proj1, output of fc_out)
DF = 1024        # d_ff / attention total feature dim
CH = 8           # feature chunks of DF
DK = 128
SCALE = 1.0 / 16.0   # 1/sqrt(2*DK)

_CACHE = {}


def _r(ap):
    return ap.bitcast(F32R)


def mm(nc, out, lhsT, rhs, start, stop):
    nc.tensor.matmul(out, _r(lhsT), _r(rhs), start=start, stop=stop)


def mmb(nc, out, lhsT, rhs, start, stop):
    nc.tensor.matmul(out, lhsT, rhs, start=start, stop=stop)


def build(loop_n=1, mode='full'):
    nc = bacc.Bacc("TRN2", target_bir_lowering=False, debug=False)

    xT = nc.dram_tensor("xT", [DIN, R], BF16, kind="ExternalInput")
    xkT = nc.dram_tensor("xkT", [DIN, R], BF16, kind="ExternalInput")
    w1_names = ["wvt", "wkt", "wqt", "wkkt", "wkqt"]
    w1 = {n: nc.dram_tensor(n, [DIN, DF], BF16, kind="ExternalInput")
          for n in w1_names}
    w2_names = ["wq2t", "wk2t", "wqft", "wkft"]
    w2 = {n: nc.dram_tensor(n, [DF, DF], BF16, kind="ExternalInput")
          for n in w2_names}
    woutt = nc.dram_tensor("woutt", [DF, DIN], BF16, kind="ExternalInput")
    ones = nc.dram_tensor("ones", [128, 128], F32, kind="ExternalInput")
    b_names = ["bv", "bk", "bq", "bkk", "bkq", "bq2", "bk2", "bqf", "bkf"]
    biases = {n: nc.dram_tensor(n, [DF], F32, kind="ExternalInput")
              for n in b_names}
    bpack = nc.dram_tensor("bpack", [128, 64], F32, kind="ExternalInput")
    out = nc.dram_tensor("out", [R, DIN], F32, kind="ExternalOutput")

    with tile.TileContext(nc) as tc:
        with (
            tc.tile_pool(name="xp", bufs=1) as xp,          # xT/xkT: 2x8KB
            tc.tile_pool(name="wp", bufs=28) as wp,         # weight chunks 28x2KB
            tc.tile_pool(name="wop", bufs=1) as wop,        # fc_out weights 16KB
            tc.tile_pool(name="t1p", bufs=2) as t1p,        # stage1 out: 2x8KB
            tc.tile_pool(name="t2p", bufs=4) as t2p,        # stage2 outs: 4x8KB
            tc.tile_pool(name="vp", bufs=1) as vp,          # v: 8KB
            tc.tile_pool(name="ep", bufs=2) as ep,          # expT: 2x16KB
            tc.tile_pool(name="smp", bufs=1) as smp,        # softmax work
            tc.tile_pool(name="op", bufs=2) as op,          # outT/final
            tc.tile_pool(name="bp", bufs=1) as bp,          # biases
            tc.tile_pool(name="psA", bufs=3, space="PSUM") as psA,   # 3 banks
            tc.tile_pool(name="psB", bufs=2, space="PSUM") as psB,   # 4 banks
            tc.tile_pool(name="psD", bufs=1, space="PSUM") as psD,   # 1 bank
        ):
            from contextlib import nullcontext
            loop_ctx = tc.For_i(0, loop_n, 1) if loop_n > 1 else nullcontext()
            with loop_ctx:
                # ---- load inputs -------------------------------------------
                xt_sb = xp.tile([128, 4, R], BF16, tag="xt")
                xkt_sb = xp.tile([128, 4, R], BF16, tag="xkt")
                LOAD = mode != "compute"

                def tok(out_ap, in_row):
                    # token DMA: writes the tile (allocates its slot) cheaply
                    nc.sync.dma_start(out_ap[0:1, 0:8], in_row)

                def load_w_chunks(dram, nchunks):
                    tiles = []
                    for kc in range(nchunks):
                        t = wp.tile([128, DF], BF16, tag="w")
                        if LOAD:
                            nc.sync.dma_start(t[:],
                                              dram[kc * 128:(kc + 1) * 128, :])
                        else:
                            tok(t[:], dram[0:1, 0:8])
                        tiles.append(t)
                    return tiles

                # critical path first: x chunk 0 + v-weights, then rest of x
                if LOAD:
                    nc.gpsimd.dma_start(xt_sb[:, 0, :], xT[0:128, :])
                else:
                    nc.gpsimd.dma_start(xt_sb[0:1, 0, 0:8], xT[0:1, 0:8])
                wv_c = load_w_chunks(w1["wvt"], 4)
                if LOAD:
                    for kc in range(1, 4):
                        nc.gpsimd.dma_start(xt_sb[:, kc, :],
                                            xT[kc * 128:(kc + 1) * 128, :])
                else:
                    for kc in range(1, 4):
                        nc.gpsimd.dma_start(xt_sb[0:1, kc, 0:8], xT[0:1, 0:8])

                # bv broadcast to all partitions for the row-major v layout
                bvb = bp.tile([128, DF], F32, tag="bvb")
                if LOAD:
                    nc.gpsimd.dma_start(
                        bvb[:], bass.AP(biases["bv"], 0, [[0, 128], [1, DF]]))
                else:
                    nc.gpsimd.dma_start(bvb[0:1, 0:8], biases["bv"][0:8])
                # packed per-chunk bias columns: bpack[:, i*8+c] = b_i[c*128+p]
                bpack_sb = bp.tile([128, 64], F32, tag="bpack")
                if LOAD:
                    nc.gpsimd.dma_start(bpack_sb[:], bpack[:])
                else:
                    nc.gpsimd.dma_start(bpack_sb[0:1, 0:8], bpack[0:1, 0:8])
                b_sb = {n: bpack_sb[:, i * 8:(i + 1) * 8]
                        for i, n in enumerate(b_names[1:])}
                ones_sb = bp.tile([128, 128], F32, tag="ones")
                nc.sync.dma_start(_r(ones_sb[:]), _r(ones[:]))
                # PE warmup: matmuls on the first-arrived x chunk fill the
                # head weight-DMA wait and warm the HAM clock gate
                warm_ps = psB.tile([128, 1024], F32, tag="attp")
                for i in range(16):
                    mmb(nc, warm_ps[:, 0:512], xt_sb[:, 0, 0:128],
                        xt_sb[:, 0, :], start=True, stop=True)
                if LOAD:
                    for kc in range(4):
                        nc.gpsimd.dma_start(xkt_sb[:, kc, :],
                                            xkT[kc * 128:(kc + 1) * 128, :])
                else:
                    for kc in range(4):
                        nc.gpsimd.dma_start(xkt_sb[0:1, kc, 0:8],
                                            xkT[0:1, 0:8])

                # ---- v = x @ Wv.T + bv   (row-major: 128 rows x 1024 feat) ------
                v_sb = vp.tile([128, NB * DF], BF16, tag="v")
                for rt in range(NB):
                    for n in range(2):
                        ps = psA.tile([128, 512], F32, tag="psA")
                        for kc in range(4):
                            mmb(nc, ps[:],
                                xt_sb[:, kc, rt * 128:(rt + 1) * 128],
                                wv_c[kc][:, n * 512:(n + 1) * 512],
                                start=(kc == 0), stop=(kc == 3))
                        nc.vector.tensor_add(
                            v_sb[:, rt * DF + n * 512: rt * DF + (n + 1) * 512],
                            ps[:], bvb[:, n * 512:(n + 1) * 512])

                # ---- k/q/kf/qf: two-stage projections, feature-major ------
                # Paired emission (s1a, s1b, s2a, s2b): by the time a stage-2
                # reaches the in-order PE queue, its stage-1 copies and w2
                # DMAs have had a full stage of slack.
                def stage1(xsrc, w1n, b1n):
                    w1_c = load_w_chunks(w1[w1n], 4)
                    t1 = t1p.tile([128, CH, R], BF16, tag="t1")
                    for dc in range(CH):
                        ps = psA.tile([128, 512], F32, tag="psA")
                        for kc in range(4):
                            mmb(nc, ps[:],
                                w1_c[kc][:, dc * 128:(dc + 1) * 128],
                                xsrc[:, kc, :],
                                start=(kc == 0), stop=(kc == 3))
                        # bias fused into PSUM->SBUF move; alternate engines
                        if dc % 2 == 0:
                            nc.scalar.activation(t1[:, dc, :], ps[:], IDENT,
                                                 bias=b_sb[b1n][:, dc:dc + 1])
                        else:
                            nc.vector.tensor_scalar_add(t1[:, dc, :], ps[:],
                                                        b_sb[b1n][:, dc:dc + 1])
                    return t1

                def stage2(name, w2n, b2n, t1):
                    w2_c = load_w_chunks(w2[w2n], 8)
                    t2 = t2p.tile([128, CH, R], BF16, tag="t2")
                    t2_sb[name] = t2
                    for dc in range(CH):
                        ps = psA.tile([128, 512], F32, tag="psA")
                        for kc in range(CH):
                            mmb(nc, ps[:],
                                w2_c[kc][:, dc * 128:(dc + 1) * 128],
                                t1[:, kc, :],
                                start=(kc == 0), stop=(kc == 7))
                        if dc % 2 == 0:
                            nc.vector.tensor_scalar_add(t2[:, dc, :], ps[:],
                                                        b_sb[b2n][:, dc:dc + 1])
                        else:
                            nc.scalar.activation(t2[:, dc, :], ps[:], IDENT,
                                                 bias=b_sb[b2n][:, dc:dc + 1])

                t2_sb = {}
                for (na, xa, w1a, b1a, w2a, b2a), (nb, xb, w1b, b1b, w2b, b2b) in [
                    (("k", xt_sb, "wkt", "bk", "wk2t", "bk2"),
                     ("q", xt_sb, "wqt", "bq", "wq2t", "bq2")),
                    (("kf", xkt_sb, "wkkt", "bkk", "wkft", "bkf"),
                     ("qf", xkt_sb, "wkqt", "bkq", "wqft", "bqf")),
                ]:
                    t1a = stage1(xa, w1a, b1a)
                    t1b = stage1(xb, w1b, b1b)
                    stage2(na, w2a, b2a, t1a)
                    stage2(nb, w2b, b2b, t1b)

                k2, q2 = t2_sb["k"], t2_sb["q"]
                kf2, qf2 = t2_sb["kf"], t2_sb["qf"]

                # fc_out weights (needed from the first block's tail onward)
                wo_sb = wop.tile([128, CH, DIN], BF16, tag="wo")
                if LOAD:
                    nc.sync.dma_start(
                        wo_sb[:], woutt.rearrange("(c p) j -> p c j", p=128))
                else:
                    nc.sync.dma_start(wo_sb[0:1, 0, 0:8], woutt[0:1, 0:8])

                # ---- attention + fc_out, software-pipelined over blocks ----
                # Engine queues are in-order FIFOs, so emission order = PE
                # order: emit att(b) before out-path(b-1) so the PE never
                # waits on block b-1's exp/softmax chain.
                exp_t = {}
                bcast_t = {}

                def att_block(b):
                    bs = b * 128
                    expT = ep.tile([128, CH, 1024], BF16, tag="expT")
                    exp_t[b] = expT
                    sumc = smp.tile([128, 1024], F32, tag="sumc")
                    for cj in range(CH):
                        attp = psB.tile([128, 1024], F32, tag="attp")
                        for nh in range(2):
                            mmb(nc, attp[:, nh * 512:(nh + 1) * 512],
                                k2[:, cj, bs:bs + 128],
                                q2[:, nh * 4:(nh + 1) * 4, bs:bs + 128],
                                start=True, stop=False)
                        for nh in range(2):
                            mmb(nc, attp[:, nh * 512:(nh + 1) * 512],
                                kf2[:, cj, bs:bs + 128],
                                qf2[:, nh * 4:(nh + 1) * 4, bs:bs + 128],
                                start=False, stop=True)
                        # exp((att+att_fei)/16): PSUM -> SBUF on ACT
                        nc.scalar.activation(expT[:, cj, :], attp[:], EXP,
                                             scale=SCALE)
                        # incremental column-sum accumulation on DVE
                        if cj == 0:
                            nc.vector.tensor_copy(_r(sumc[:]), expT[:, 0, :])
                        else:
                            nc.vector.tensor_add(_r(sumc[:]), sumc[:],
                                                 expT[:, cj, :])
                    return sumc

                def sum_block(b, sumc):
                    # partition reduce via ones-matmul (DVE can't cross
                    # partitions); 2 tiny PE matmuls
                    recip = smp.tile([1, 1024], F32, tag="recip")
                    for nh in range(2):
                        cs = psD.tile([1, 512], F32, tag="cs")
                        mm(nc, cs[:], ones_sb[:, 0:1],
                           sumc[:, nh * 512:(nh + 1) * 512],
                           start=True, stop=True)
                        nc.vector.reciprocal(
                            recip[0:1, nh * 512:(nh + 1) * 512], cs[:])
                    bcastR = smp.tile([128, 1024], F32, tag="bcastR")
                    nc.gpsimd.partition_broadcast(bcastR[:], recip[:])
                    bcast_t[b] = bcastR

                def out_block(b):
                    bs = b * 128
                    expT = exp_t.pop(b)
                    bcastR = bcast_t.pop(b)
                    # out'.T = sum_cj Vcj-weighted exp: (128 d, 1024 i')
                    outp0 = psA.tile([128, 512], F32, tag="psA")
                    outp1 = psA.tile([128, 512], F32, tag="psA")
                    outp = [outp0, outp1]
                    for cj in range(CH):
                        for nh in range(2):
                            mmb(nc, outp[nh][:],
                                v_sb[:, b * DF + cj * 128:
                                     b * DF + (cj + 1) * 128],
                                expT[:, cj, nh * 512:(nh + 1) * 512],
                                start=(cj == 0), stop=(cj == 7))
                    outT = op.tile([128, 1024], BF16, tag="outT")
                    for nh in range(2):
                        nc.vector.tensor_mul(outT[:, nh * 512:(nh + 1) * 512],
                                             outp[nh][:],
                                             bcastR[:, nh * 512:(nh + 1) * 512])
                    # fc_out for this block: (128 rows, 512 dm), bout on host
                    fcp = psA.tile([128, 512], F32, tag="psA")
                    for c in range(CH):
                        mmb(nc, fcp[:],
                            outT[:, c * 128:(c + 1) * 128],
                            wo_sb[:, c, :],
                            start=(c == 0), stop=(c == 7))
                    final = op.tile([128, 512], F32, tag="final")
                    nc.vector.tensor_copy(final[:], fcp[:])
                    nc.gpsimd.dma_start(out[bs:bs + 128, :], final[:])

                for b in range(NB):
                    sumc = att_block(b)
                    if b == 0:
                        sum_block(b, sumc)
                    if b >= 1:
                        out_block(b - 1)
                        sum_block(b, sumc)
                out_block(NB - 1)

    nc.compile()
    return nc


def build_dma(loop_n=1):
    """DMA-only probe: all input loads + output stores, no compute."""
    nc = bacc.Bacc("TRN2", target_bir_lowering=False, debug=False)
    xT = nc.dram_tensor("xT", [DIN, R], BF16, kind="ExternalInput")
    xkT = nc.dram_tensor("xkT", [DIN, R], BF16, kind="ExternalInput")
    w1_names = ["wvt", "wkt", "wqt", "wkkt", "wkqt"]
    w1 = {n: nc.dram_tensor(n, [DIN, DF], BF16, kind="ExternalInput")
          for n in w1_names}
    w2_names = ["wq2t", "wk2t", "wqft", "wkft"]
    w2 = {n: nc.dram_tensor(n, [DF, DF], BF16, kind="ExternalInput")
          for n in w2_names}
    woutt = nc.dram_tensor("woutt", [DF, DIN], BF16, kind="ExternalInput")
    ones = nc.dram_tensor("ones", [128, 128], F32, kind="ExternalInput")
    b_names = ["bv", "bk", "bq", "bkk", "bkq", "bq2", "bk2", "bqf", "bkf"]
    biases = {n: nc.dram_tensor(n, [DF], F32, kind="ExternalInput")
              for n in b_names}
    out = nc.dram_tensor("out", [R, DIN], F32, kind="ExternalOutput")
    with tile.TileContext(nc) as tc:
        with (
            tc.tile_pool(name="xp", bufs=1) as xp,
            tc.tile_pool(name="wp", bufs=20) as wp,
            tc.tile_pool(name="wop", bufs=1) as wop,
            tc.tile_pool(name="bp", bufs=1) as bp,
            tc.tile_pool(name="op", bufs=1) as op,
        ):
            from contextlib import nullcontext
            loop_ctx = tc.For_i(0, loop_n, 1) if loop_n > 1 else nullcontext()
            with loop_ctx:
                xt_sb = xp.tile([128, 4, R], BF16, tag="xt")
                xkt_sb = xp.tile([128, 4, R], BF16, tag="xkt")
                for kc in range(4):
                    nc.sync.dma_start(xt_sb[:, kc, :],
                                      xT[kc * 128:(kc + 1) * 128, :])
                    nc.sync.dma_start(xkt_sb[:, kc, :],
                                      xkT[kc * 128:(kc + 1) * 128, :])
                for n in w1_names:
                    for kc in range(4):
                        t = wp.tile([128, DF], BF16, tag="w")
                        nc.sync.dma_start(t[:],
                                          w1[n][kc * 128:(kc + 1) * 128, :])
                for n in w2_names:
                    for kc in range(8):
                        t = wp.tile([128, DF], BF16, tag="w")
                        nc.sync.dma_start(t[:],
                                          w2[n][kc * 128:(kc + 1) * 128, :])
                wo_d = wop.tile([128, CH, DIN], BF16, tag="wo")
                nc.sync.dma_start(
                    wo_d[:], woutt.rearrange("(c p) j -> p c j", p=128))
                bvb = bp.tile([128, DF], F32, tag="bvb")
                nc.sync.dma_start(
                    bvb[:], bass.AP(biases["bv"], 0, [[0, 128], [1, DF]]))
                for n in b_names[1:]:
                    t = bp.tile([128, CH], F32, tag=f"b_{n}")
                    nc.sync.dma_start(t[:],
                                      biases[n].rearrange("(c p) -> p c", p=128))
                ones_sb = bp.tile([128, 128], F32, tag="ones")
                nc.sync.dma_start(ones_sb[:], ones[:])
                fin_d = op.tile([128, 512], F32, tag="final")
                nc.gpsimd.memset(fin_d[:], 0.0)
                for b in range(NB):
                    nc.sync.dma_start(out[b * 128:(b + 1) * 128, :], fin_d[:])
    nc.compile()
    return nc


def round_fp32r(x):
    """fp32 -> fp32r (round-to-nearest-even at 11 mantissa bits)."""
    u = np.ascontiguousarray(x, dtype=np.float32).view(np.uint32)
    lsb = (u >> np.uint32(12)) & np.uint32(1)
    u = (u + np.uint32(0x7FF) + lsb) & np.uint32(0xFFFFF000)
    return u.view(np.float32)


def prep_in_maps(inputs):
    x = np.ascontiguousarray(inputs["x"], dtype=np.float32)
    xk = np.ascontiguousarray(inputs["x_knowledge"], dtype=np.float32)
    B, L, DM = x.shape
    x_flat = x.reshape(B * L, DM)
    xk_flat = xk.reshape(B * L, DM)
    f32 = np.float32

    import ml_dtypes

    def wt(name):
        return np.ascontiguousarray(inputs[name].T).astype(ml_dtypes.bfloat16)

    shared = {
        "wvt": wt("Wv"), "wkt": wt("Wk"), "wqt": wt("Wq"),
        "wkkt": wt("Wkk"), "wkqt": wt("Wkq"),
        "wq2t": wt("Wq2"), "wk2t": wt("Wk2"),
        "wqft": wt("Wqf"), "wkft": wt("Wkf"),
        "woutt": wt("Wout"),
        "ones": np.ones((128, 128), dtype=f32),
        "bv": np.asarray(inputs["bv"], dtype=f32),
        "bk": np.asarray(inputs["bk"], dtype=f32),
        "bq": np.asarray(inputs["bq"], dtype=f32),
        "bkk": np.asarray(inputs["bkk"], dtype=f32),
        "bkq": np.asarray(inputs["bkq"], dtype=f32),
        "bq2": np.asarray(inputs["bq2"], dtype=f32),
        "bk2": np.asarray(inputs["bk2"], dtype=f32),
        "bqf": np.asarray(inputs["bqf"], dtype=f32),
        "bkf": np.asarray(inputs["bkf"], dtype=f32),
    }
    bpack = np.zeros((128, 64), dtype=f32)
    for i, n in enumerate(["bk", "bq", "bkk", "bkq", "bq2", "bk2", "bqf", "bkf"]):
        bpack[:, i * 8:(i + 1) * 8] = np.asarray(inputs[n], dtype=f32).reshape(8, 128).T
    shared["bpack"] = bpack
    in_maps = []
    for c in range(N_CORES):
        sl = slice(c * R, (c + 1) * R)
        m = dict(shared)
        m["xT"] = np.ascontiguousarray(x_flat[sl].T).astype(ml_dtypes.bfloat16)
        m["xkT"] = np.ascontiguousarray(xk_flat[sl].T).astype(ml_dtypes.bfloat16)
        in_maps.append(m)
    return in_maps


def kernel(**inputs):
    if "nc" not in _CACHE:
        _CACHE["nc"] = build()
    nc = _CACHE["nc"]
    in_maps = prep_in_maps(inputs)
    B, L, DM = inputs["x"].shape
    f32 = np.float32

    res = run_bass_kernel_spmd(nc, in_maps, core_ids=list(range(N_CORES)))
    _CACHE["last_results"] = res
    out_flat = np.concatenate([res.results[c]["out"] for c in range(N_CORES)],
                              axis=0)
    out_flat = out_flat + np.asarray(inputs["bout"], dtype=f32)[None, :]
    return out_flat.reshape(B, L, DM).astype(np.float32)


if __name__ == "__main__":
    if "--compile-only" in sys.argv:
        import tempfile
        from concourse.bass_utils import compile_bass_kernel
        nc = build()
        print("bacc build OK; walrus-compiling...")
        print("OK:", compile_bass_kernel(nc, tempfile.mkdtemp()))



# revision 2
# speedup vs baseline: 1.3703x; 1.3703x over previous
"""Trainium2 Bass kernel for nn_Aliformer (dense transformer w/ knowledge attention).

Math (reference, B=4 L=1024 DM=512 DF=1024 H=8 DK=128):
  v/k/q       = x @ {Wv,Wk,Wq}.T + b            (B,L,1024)
  k_fei/q_fei = x_knowledge @ {Wkk,Wkq}.T + b   (B,L,1024)
  q,k,qf,kf   = second linear (1024->1024), then torch-style reshape
                (B,L,1024)->(B,8,1024,128) WITHOUT transpose.
  att  = (q@k^T + qf@kf^T)/sqrt(256); score = softmax(att)
  out  = (score @ v-reshaped) -> (B,L,1024); final = out @ Wout.T + bout

Key structural fact: the no-transpose reshape means head h of batch b only
touches rows [h*128,(h+1)*128) of the flattened (4096, ·) activations, so the
whole network decomposes into 32 independent 128-row blocks. Each of the 8
cores processes 4 contiguous blocks (512 rows) with zero communication.

Within the attention of a block, row index l' = 8r + c (r = row-in-block,
c = feature chunk). We compute everything in the permuted order i' = c*128+r
(softmax is permutation-invariant over the full axis).

v2 (fp8 DoubleRow): every matmul that only feeds the softmax LOGITS runs in
fp8e4m3 with perf_mode=DoubleRow (2 fp8 K-slices packed per PE cell -> one
K=256 contraction per instruction):
  - stage-1 k/q/kf/qf projections (x quantized to fp8; the v projection keeps
    x in bf16 since v errors pass straight to the output),
  - all four stage-2 projections (t1 stored fp8),
  - the attention QK^T matmuls, pairing (k2,q2) with (kf2,qf2) so
    att+att_fei accumulate inside one DoubleRow instruction.
Score@V / fc_out / v stay bf16 (fp8 there fails the 2e-2 gate; emulated
end-to-end error of this split is 1.4e-2).

Stage-1 biases are folded into stage-2 on the host (b2' = b2 + W2 @ b1), so
stage-1 PSUM->SBUF moves are pure quantizing copies. Stage-2 outputs write
directly into paired attention layouts kk/qq[128d, block, {base,fei}, i'] so
the DoubleRow attention operands are natural 3D slices. Softmax column sums
accumulate in bf16 (DVE 2x mode); partition reduction via ones-matmul;
bcast recip via gpsimd; division folded into the attention-output move.
bout is added on the host; bv on device (DMA-broadcast tile).

Weights are host-prepacked: w1 fp8 [128, 2pair, 2, 1024], w2 fp8
[128, 4pair, 2, 1024] where element [p, pr, i, dc*128+m] = W.T[(2pr+i)*128+p,
dc*128+m]; wv/wout stay bf16.
"""

import sys

for _p in ("/opt/trn_rl_repo", "/root/.axon_site/_ro/trn_rl_repo"):
    if _p not in sys.path:
        sys.path.insert(0, _p)

import numpy as np

import concourse.bass as bass
import concourse.mybir as mybir
import concourse.tile as tile
from concourse import bacc
from concourse.bass_utils import run_bass_kernel_spmd

F32 = mybir.dt.float32
BF16 = mybir.dt.bfloat16
F8 = mybir.dt.float8e4
DR = mybir.MatmulPerfMode.DoubleRow
EXP = mybir.ActivationFunctionType.Exp
IDENT = mybir.ActivationFunctionType.Identity

N_CORES = 8
R = 512          # rows per core
NB = 4           # 128-row blocks per core
DIN = 512        # model dim (input of proj1, output of fc_out)
DF = 1024        # d_ff / attention total feature dim
CH = 8           # feature chunks of DF
DK = 128
SCALE = 1.0 / 16.0   # 1/sqrt(2*DK)

_CACHE = {}


def build(loop_n=1, mode='full'):
    nc = bacc.Bacc("TRN2", target_bir_lowering=False, debug=False)

    xT = nc.dram_tensor("xT", [DIN, R], BF16, kind="ExternalInput")
    xT8 = nc.dram_tensor("xT8", [DIN, R], F8, kind="ExternalInput")
    xkT8 = nc.dram_tensor("xkT8", [DIN, R], F8, kind="ExternalInput")
    wvt = nc.dram_tensor("wvt", [DIN, DF], BF16, kind="ExternalInput")
    w1_names = ["wkt8", "wqt8", "wkkt8", "wkqt8"]
    w1 = {n: nc.dram_tensor(n, [128, 2, 2, DF], F8, kind="ExternalInput")
          for n in w1_names}
    w2_names = ["wk2t8", "wq2t8", "wkft8", "wqft8"]
    w2 = {n: nc.dram_tensor(n, [128, 4, 2, DF], F8, kind="ExternalInput")
          for n in w2_names}
    woutt = nc.dram_tensor("woutt", [DF, DIN], BF16, kind="ExternalInput")
    ones = nc.dram_tensor("ones", [128, 128], BF16, kind="ExternalInput")
    bv = nc.dram_tensor("bv", [DF], F32, kind="ExternalInput")
    b2pack = nc.dram_tensor("b2pack", [128, 32], F32, kind="ExternalInput")
    out = nc.dram_tensor("out", [R, DIN], F32, kind="ExternalOutput")

    with tile.TileContext(nc) as tc:
        with (
            tc.tile_pool(name="xp", bufs=1) as xp,          # xT/x8/xk8
            tc.tile_pool(name="wvp", bufs=4) as wvp,        # wv chunks
            tc.tile_pool(name="w1p", bufs=4) as w1p,        # fp8 w1 packs
            tc.tile_pool(name="w2p", bufs=4) as w2p,        # fp8 w2 packs
            tc.tile_pool(name="wop", bufs=1) as wop,        # fc_out weights
            tc.tile_pool(name="t1p", bufs=2) as t1p,        # stage1 out fp8
            tc.tile_pool(name="t2p", bufs=2) as t2p,        # kk/qq fp8
            tc.tile_pool(name="vp", bufs=1) as vp,          # v bf16
            tc.tile_pool(name="ep", bufs=2) as ep,          # expT bf16
            tc.tile_pool(name="smp", bufs=1) as smp,        # softmax work
            tc.tile_pool(name="op", bufs=2) as op,          # outT/final
            tc.tile_pool(name="bp", bufs=1) as bp,          # biases
            tc.tile_pool(name="psA", bufs=3, space="PSUM") as psA,   # 3 banks
            tc.tile_pool(name="psB", bufs=2, space="PSUM") as psB,   # 4 banks
            tc.tile_pool(name="psD", bufs=1, space="PSUM") as psD,   # 1 bank
        ):
            from contextlib import nullcontext
            loop_ctx = tc.For_i(0, loop_n, 1) if loop_n > 1 else nullcontext()
            with loop_ctx:
                # ---- load inputs -------------------------------------------
                xt_sb = xp.tile([128, 4, R], BF16, tag="xt")
                x8_sb = xp.tile([128, 4, R], F8, tag="x8")
                xk8_sb = xp.tile([128, 4, R], F8, tag="xk8")
                LOAD = mode != "compute"

                def tok(out_ap, in_row):
                    # token DMA: writes the tile (allocates its slot) cheaply
                    nc.sync.dma_start(out_ap, in_row)

                # critical path first: x chunk 0 + v-weights, then rest of x
                if LOAD:
                    nc.gpsimd.dma_start(xt_sb[:, 0, :], xT[0:128, :])
                else:
                    nc.gpsimd.dma_start(xt_sb[0:1, 0, 0:8], xT[0:1, 0:8])
                wv_c = []
                for kc in range(4):
                    t = wvp.tile([128, DF], BF16, tag="wv")
                    if LOAD:
                        nc.sync.dma_start(t[:], wvt[kc * 128:(kc + 1) * 128, :])
                    else:
                        tok(t[0:1, 0:8], wvt[0:1, 0:8])
                    wv_c.append(t)
                if LOAD:
                    for kc in range(1, 4):
                        nc.gpsimd.dma_start(xt_sb[:, kc, :],
                                            xT[kc * 128:(kc + 1) * 128, :])
                    for kc in range(4):
                        nc.gpsimd.dma_start(x8_sb[:, kc, :],
                                            xT8[kc * 128:(kc + 1) * 128, :])
                else:
                    for kc in range(1, 4):
                        nc.gpsimd.dma_start(xt_sb[0:1, kc, 0:8], xT[0:1, 0:8])
                    for kc in range(4):
                        nc.gpsimd.dma_start(x8_sb[0:1, kc, 0:8], xT8[0:1, 0:8])

                # bv broadcast to all partitions for the row-major v layout
                bvb = bp.tile([128, DF], F32, tag="bvb")
                if LOAD:
                    nc.gpsimd.dma_start(
                        bvb[:], bass.AP(bv, 0, [[0, 128], [1, DF]]))
                else:
                    nc.gpsimd.dma_start(bvb[0:1, 0:8], bv[0:8])
                # packed folded stage2 biases: [:, i*8+dc] = b2'_i[dc*128+p]
                b2p_sb = bp.tile([128, 32], F32, tag="b2p")
                if LOAD:
                    nc.gpsimd.dma_start(b2p_sb[:], b2pack[:])
                else:
                    nc.gpsimd.dma_start(b2p_sb[0:1, 0:8], b2pack[0:1, 0:8])
                ones_sb = bp.tile([128, 128], BF16, tag="ones")
                nc.sync.dma_start(ones_sb[:], ones[:])
                # PE warmup: matmuls on the first-arrived x chunk fill the
                # head weight-DMA wait and warm the HAM clock gate
                warm_ps = psB.tile([128, 1024], F32, tag="attp")
                for i in range(16):
                    nc.tensor.matmul(warm_ps[:, 0:512], xt_sb[:, 0, 0:128],
                                     xt_sb[:, 0, :], start=True, stop=True)
                if LOAD:
                    for kc in range(4):
                        nc.gpsimd.dma_start(xk8_sb[:, kc, :],
                                            xkT8[kc * 128:(kc + 1) * 128, :])
                else:
                    for kc in range(4):
                        nc.gpsimd.dma_start(xk8_sb[0:1, kc, 0:8],
                                            xkT8[0:1, 0:8])

                # ---- v = x @ Wv.T + bv (bf16, row-major 128 x 1024) --------
                v_sb = vp.tile([128, NB * DF], BF16, tag="v")
                for rt in range(NB):
                    for n in range(2):
                        ps = psA.tile([128, 512], F32, tag="psA")
                        for kc in range(4):
                            nc.tensor.matmul(
                                ps[:],
                                xt_sb[:, kc, rt * 128:(rt + 1) * 128],
                                wv_c[kc][:, n * 512:(n + 1) * 512],
                                start=(kc == 0), stop=(kc == 3))
                        nc.vector.tensor_add(
                            v_sb[:, rt * DF + n * 512: rt * DF + (n + 1) * 512],
                            ps[:], bvb[:, n * 512:(n + 1) * 512])

                # paired attention-layout tiles: [128 d, block, {base,fei}, i']
                kk = t2p.tile([128, NB, 2, DF], F8, tag="kk")
                qq = t2p.tile([128, NB, 2, DF], F8, tag="qq")

                # ---- k/q/kf/qf: fp8 DoubleRow two-stage projections --------
                def stage1(x8src, w1n):
                    w1_sb = w1p.tile([128, 2, 2, DF], F8, tag="w1")
                    if LOAD:
                        nc.sync.dma_start(w1_sb[:], w1[w1n][:])
                    else:
                        tok(w1_sb[0:1, 0, 0, 0:8], w1[w1n][0:1, 0, 0, 0:8])
                    t1 = t1p.tile([128, CH, R], F8, tag="t1")
                    for dc in range(CH):
                        ps = psA.tile([128, 512], F32, tag="psA")
                        for pr in range(2):
                            nc.tensor.matmul(
                                ps[:], w1_sb[:, pr, :, dc * 128:(dc + 1) * 128],
                                x8src[:, 2 * pr:2 * pr + 2, :],
                                start=(pr == 0), stop=(pr == 1), perf_mode=DR)
                        # bias folded into stage2: pure quantizing copy
                        if dc % 2 == 0:
                            nc.scalar.activation(t1[:, dc, :], ps[:], IDENT)
                        else:
                            nc.vector.tensor_copy(t1[:, dc, :], ps[:])
                    return t1

                def stage2(dst, iq, bidx, w2n, t1):
                    w2_sb = w2p.tile([128, 4, 2, DF], F8, tag="w2")
                    if LOAD:
                        nc.sync.dma_start(w2_sb[:], w2[w2n][:])
                    else:
                        tok(w2_sb[0:1, 0, 0, 0:8], w2[w2n][0:1, 0, 0, 0:8])
                    for dc in range(CH):
                        ps = psA.tile([128, 512], F32, tag="psA")
                        for pr in range(4):
                            nc.tensor.matmul(
                                ps[:], w2_sb[:, pr, :, dc * 128:(dc + 1) * 128],
                                t1[:, 2 * pr:2 * pr + 2, :],
                                start=(pr == 0), stop=(pr == 3), perf_mode=DR)
                        out_ap = dst[:, :, iq, dc * 128:(dc + 1) * 128]
                        bcol = b2p_sb[:, bidx * 8 + dc:bidx * 8 + dc + 1]
                        if dc % 2 == 0:
                            nc.vector.tensor_scalar_add(out_ap, ps[:], bcol)
                        else:
                            nc.scalar.activation(out_ap, ps[:], IDENT,
                                                 bias=bcol)

                # paired emission (s1a, s1b, s2a, s2b) for write->read slack
                for (xa, w1a, dsta, iqa, bia, w2a), (xb, w1b, dstb, iqb, bib, w2b) in [
                    ((x8_sb, "wkt8", kk, 0, 0, "wk2t8"),
                     (x8_sb, "wqt8", qq, 0, 1, "wq2t8")),
                    ((xk8_sb, "wkkt8", kk, 1, 2, "wkft8"),
                     (xk8_sb, "wkqt8", qq, 1, 3, "wqft8")),
                ]:
                    t1a = stage1(xa, w1a)
                    t1b = stage1(xb, w1b)
                    stage2(dsta, iqa, bia, w2a, t1a)
                    stage2(dstb, iqb, bib, w2b, t1b)

                # fc_out weights (needed from the first block's tail onward)
                wo_sb = wop.tile([128, CH, DIN], BF16, tag="wo")
                if LOAD:
                    nc.sync.dma_start(
                        wo_sb[:], woutt.rearrange("(c p) j -> p c j", p=128))
                else:
                    nc.sync.dma_start(wo_sb[0:1, 0, 0:8], woutt[0:1, 0:8])

                # ---- attention + fc_out, software-pipelined over blocks ----
                # Engine queues are in-order FIFOs, so emission order = PE
                # order: emit att(b) before out-path(b-1) so the PE never
                # waits on block b-1's exp/softmax chain.
                exp_t = {}
                bcast_t = {}

                def att_block(b):
                    expT = ep.tile([128, CH, 1024], BF16, tag="expT")
                    exp_t[b] = expT
                    sumc = smp.tile([128, 1024], BF16, tag="sumc")
                    for cj in range(CH):
                        attp = psB.tile([128, 1024], F32, tag="attp")
                        for nh in range(2):
                            # one DoubleRow mm = q.k + qf.kf (K=256)
                            nc.tensor.matmul(
                                attp[:, nh * 512:(nh + 1) * 512],
                                kk[:, b, :, cj * 128:(cj + 1) * 128],
                                qq[:, b, :, nh * 512:(nh + 1) * 512],
                                start=True, stop=True, perf_mode=DR)
                        # exp((att+att_fei)/16): PSUM -> SBUF on ACT
                        nc.scalar.activation(expT[:, cj, :], attp[:], EXP,
                                             scale=SCALE)
                        # bf16 column-sum accumulation on DVE (2x mode)
                        if cj == 1:
                            nc.vector.tensor_add(sumc[:], expT[:, 0, :],
                                                 expT[:, 1, :])
                        elif cj > 1:
                            nc.vector.tensor_add(sumc[:], sumc[:],
                                                 expT[:, cj, :])
                    return sumc

                def sum_block(b, sumc):
                    # partition reduce via ones-matmul (DVE can't cross
                    # partitions); 2 tiny PE matmuls
                    recip = smp.tile([1, 1024], F32, tag="recip")
                    for nh in range(2):
                        cs = psD.tile([1, 512], F32, tag="cs")
                        nc.tensor.matmul(cs[:], ones_sb[:, 0:1],
                                         sumc[:, nh * 512:(nh + 1) * 512],
                                         start=True, stop=True)
                        nc.vector.reciprocal(
                            recip[0:1, nh * 512:(nh + 1) * 512], cs[:])
                    bcastR = smp.tile([128, 1024], F32, tag="bcastR")
                    nc.gpsimd.partition_broadcast(bcastR[:], recip[:])
                    bcast_t[b] = bcastR

                def out_block(b):
                    bs = b * 128
                    expT = exp_t.pop(b)
                    bcastR = bcast_t.pop(b)
                    # out'.T = sum_cj Vcj-weighted exp: (128 d, 1024 i')
                    outp0 = psA.tile([128, 512], F32, tag="psA")
                    outp1 = psA.tile([128, 512], F32, tag="psA")
                    outp = [outp0, outp1]
                    for cj in range(CH):
                        for nh in range(2):
                            nc.tensor.matmul(
                                outp[nh][:],
                                v_sb[:, b * DF + cj * 128:
                                     b * DF + (cj + 1) * 128],
                                expT[:, cj, nh * 512:(nh + 1) * 512],
                                start=(cj == 0), stop=(cj == 7))
                    outT = op.tile([128, 1024], BF16, tag="outT")
                    for nh in range(2):
                        nc.vector.tensor_mul(outT[:, nh * 512:(nh + 1) * 512],
                                             outp[nh][:],
                                             bcastR[:, nh * 512:(nh + 1) * 512])
                    # fc_out for this block: (128 rows, 512 dm), bout on host
                    fcp = psA.tile([128, 512], F32, tag="psA")
                    for c in range(CH):
                        nc.tensor.matmul(fcp[:],
                                         outT[:, c * 128:(c + 1) * 128],
                                         wo_sb[:, c, :],
                                         start=(c == 0), stop=(c == 7))
                    final = op.tile([128, 512], F32, tag="final")
                    nc.vector.tensor_copy(final[:], fcp[:])
                    nc.gpsimd.dma_start(out[bs:bs + 128, :], final[:])

                for b in range(NB):
                    sumc = att_block(b)
                    if b == 0:
                        sum_block(b, sumc)
                    if b >= 1:
                        out_block(b - 1)
                        sum_block(b, sumc)
                out_block(NB - 1)

    nc.compile()
    return nc


def build_dma(loop_n=1):
    """DMA-only probe: all input loads + output stores, no compute."""
    nc = bacc.Bacc("TRN2", target_bir_lowering=False, debug=False)
    xT = nc.dram_tensor("xT", [DIN, R], BF16, kind="ExternalInput")
    xT8 = nc.dram_tensor("xT8", [DIN, R], F8, kind="ExternalInput")
    xkT8 = nc.dram_tensor("xkT8", [DIN, R], F8, kind="ExternalInput")
    wvt = nc.dram_tensor("wvt", [DIN, DF], BF16, kind="ExternalInput")
    w1_names = ["wkt8", "wqt8", "wkkt8", "wkqt8"]
    w1 = {n: nc.dram_tensor(n, [128, 2, 2, DF], F8, kind="ExternalInput")
          for n in w1_names}
    w2_names = ["wk2t8", "wq2t8", "wkft8", "wqft8"]
    w2 = {n: nc.dram_tensor(n, [128, 4, 2, DF], F8, kind="ExternalInput")
          for n in w2_names}
    woutt = nc.dram_tensor("woutt", [DF, DIN], BF16, kind="ExternalInput")
    ones = nc.dram_tensor("ones", [128, 128], BF16, kind="ExternalInput")
    bv = nc.dram_tensor("bv", [DF], F32, kind="ExternalInput")
    b2pack = nc.dram_tensor("b2pack", [128, 32], F32, kind="ExternalInput")
    out = nc.dram_tensor("out", [R, DIN], F32, kind="ExternalOutput")
    with tile.TileContext(nc) as tc:
        with (
            tc.tile_pool(name="xp", bufs=1) as xp,
            tc.tile_pool(name="wvp", bufs=4) as wvp,
            tc.tile_pool(name="w1p", bufs=4) as w1p,
            tc.tile_pool(name="w2p", bufs=4) as w2p,
            tc.tile_pool(name="wop", bufs=1) as wop,
            tc.tile_pool(name="bp", bufs=1) as bp,
            tc.tile_pool(name="op", bufs=1) as op,
        ):
            from contextlib import nullcontext
            loop_ctx = tc.For_i(0, loop_n, 1) if loop_n > 1 else nullcontext()
            with loop_ctx:
                xt_sb = xp.tile([128, 4, R], BF16, tag="xt")
                x8_sb = xp.tile([128, 4, R], F8, tag="x8")
                xk8_sb = xp.tile([128, 4, R], F8, tag="xk8")
                for kc in range(4):
                    nc.sync.dma_start(xt_sb[:, kc, :],
                                      xT[kc * 128:(kc + 1) * 128, :])
                    nc.sync.dma_start(x8_sb[:, kc, :],
                                      xT8[kc * 128:(kc + 1) * 128, :])
                    nc.sync.dma_start(xk8_sb[:, kc, :],
                                      xkT8[kc * 128:(kc + 1) * 128, :])
                for kc in range(4):
                    t = wvp.tile([128, DF], BF16, tag="wv")
                    nc.sync.dma_start(t[:], wvt[kc * 128:(kc + 1) * 128, :])
                for n in w1_names:
                    t = w1p.tile([128, 2, 2, DF], F8, tag="w1")
                    nc.sync.dma_start(t[:], w1[n][:])
                for n in w2_names:
                    t = w2p.tile([128, 4, 2, DF], F8, tag="w2")
                    nc.sync.dma_start(t[:], w2[n][:])
                wo_d = wop.tile([128, CH, DIN], BF16, tag="wo")
                nc.sync.dma_start(
                    wo_d[:], woutt.rearrange("(c p) j -> p c j", p=128))
                bvb = bp.tile([128, DF], F32, tag="bvb")
                nc.sync.dma_start(bvb[:], bass.AP(bv, 0, [[0, 128], [1, DF]]))
                b2p_sb = bp.tile([128, 32], F32, tag="b2p")
                nc.sync.dma_start(b2p_sb[:], b2pack[:])
                ones_sb = bp.tile([128, 128], BF16, tag="ones")
                nc.sync.dma_start(ones_sb[:], ones[:])
                fin_d = op.tile([128, 512], F32, tag="final")
                nc.gpsimd.memset(fin_d[:], 0.0)
                for b in range(NB):
                    nc.sync.dma_start(out[b * 128:(b + 1) * 128, :], fin_d[:])
    nc.compile()
    return nc


def prep_in_maps(inputs):
    import ml_dtypes
    NPBF = ml_dtypes.bfloat16
    NPF8 = ml_dtypes.float8_e4m3

    x = np.ascontiguousarray(inputs["x"], dtype=np.float32)
    xk = np.ascontiguousarray(inputs["x_knowledge"], dtype=np.float32)
    B, L, DM = x.shape
    x_flat = x.reshape(B * L, DM)
    xk_flat = xk.reshape(B * L, DM)
    f32 = np.float32

    def pack_w(name, npairs):
        # [p, pr, i, m] = W.T[(2*pr+i)*128+p, m], fp8
        WT = np.ascontiguousarray(np.asarray(inputs[name], f32).T)
        arr = WT.reshape(npairs, 2, 128, DF).transpose(2, 0, 1, 3)
        return np.ascontiguousarray(arr).astype(NPF8)

    def fold_b2(w2n, b2n, b1n):
        return (np.asarray(inputs[b2n], f32)
                + np.asarray(inputs[w2n], f32) @ np.asarray(inputs[b1n], f32))

    b2pack = np.zeros((128, 32), dtype=f32)
    for i, (w2n, b2n, b1n) in enumerate([
        ("Wk2", "bk2", "bk"), ("Wq2", "bq2", "bq"),
        ("Wkf", "bkf", "bkk"), ("Wqf", "bqf", "bkq"),
    ]):
        b2pack[:, i * 8:(i + 1) * 8] = fold_b2(w2n, b2n, b1n).reshape(8, 128).T

    shared = {
        "wvt": np.ascontiguousarray(np.asarray(inputs["Wv"], f32).T).astype(NPBF),
        "wkt8": pack_w("Wk", 2), "wqt8": pack_w("Wq", 2),
        "wkkt8": pack_w("Wkk", 2), "wkqt8": pack_w("Wkq", 2),
        "wk2t8": pack_w("Wk2", 4), "wq2t8": pack_w("Wq2", 4),
        "wkft8": pack_w("Wkf", 4), "wqft8": pack_w("Wqf", 4),
        "woutt": np.ascontiguousarray(np.asarray(inputs["Wout"], f32).T).astype(NPBF),
        "ones": np.ones((128, 128), dtype=NPBF),
        "bv": np.asarray(inputs["bv"], dtype=f32),
        "b2pack": b2pack,
    }
    in_maps = []
    for c in range(N_CORES):
        sl = slice(c * R, (c + 1) * R)
        m = dict(shared)
        xTc = np.ascontiguousarray(x_flat[sl].T)
        xkTc = np.ascontiguousarray(xk_flat[sl].T)
        m["xT"] = xTc.astype(NPBF)
        m["xT8"] = xTc.astype(NPF8)
        m["xkT8"] = xkTc.astype(NPF8)
        in_maps.append(m)
    return in_maps


def kernel(**inputs):
    if "nc" not in _CACHE:
        _CACHE["nc"] = build()
    nc = _CACHE["nc"]
    in_maps = prep_in_maps(inputs)
    B, L, DM = inputs["x"].shape
    f32 = np.float32

    res = run_bass_kernel_spmd(nc, in_maps, core_ids=list(range(N_CORES)))
    _CACHE["last_results"] = res
    out_flat = np.concatenate([res.results[c]["out"] for c in range(N_CORES)],
                              axis=0)
    out_flat = out_flat + np.asarray(inputs["bout"], dtype=f32)[None, :]
    return out_flat.reshape(B, L, DM).astype(np.float32)


if __name__ == "__main__":
    if "--compile-only" in sys.argv:
        import tempfile
        from concourse.bass_utils import compile_bass_kernel
        nc = build()
        print("bacc build OK; walrus-compiling...")
        print("OK:", compile_bass_kernel(nc, tempfile.mkdtemp()))
